# revision 82
# baseline (speedup 1.0000x reference)
"""Bass/Trainium2 kernel for nn_EpisodicMemory (8-core data-parallel), v4.

Batch (16384 rows) is sharded across 8 NeuronCores (2048 rows each, 16
row-tiles of 128).  All O(weights) transforms happen on the host; all O(B)
work happens on-device.

v4 changes vs v2 (TimelineSim per-core estimate 153.9us -> 109.7us):
 - every attn-side matmul (retrieval, gate attn-part, h attn-part) runs in
   fp8-e4m3 DoubleRow: the normalized attention is quantized to fp8 with a
   x64 scale (values ~1.0, comfortably in e4m3 normal range), mem_vals /
   mem_vals@w projections stay unit-scale fp8, and the x-part weights are
   host-scaled x64 so both psum contributions share the x64 factor, which
   the sigmoid/gelu ACT un-scales for free.
 - a quarter of the h-path contraction (k < 256) also runs fp8 DoubleRow
   (end-to-end rel err 1.57e-2 vs the 2e-2 gate; half-fp8 measured
   1.96e-2 - too thin a margin).
 - the softmax chain is batched per 512-row chunk (4 tiles) and split
   into four stages (ss/sqrt, srt^T/bias-row, sim/exp/quantize,
   fp8-transposes), one stage per iteration of the PRECEDING chunk, so
   every cross-engine round-trip has a full iteration of slack and never
   convoys the in-order engine queues.  The fp8 transpose writes with the
   HW-mandated element step of 2; an ACT copy compacts it.
 - epilogue splits across iterations: matmuls + sigmoid/gelu + Pool
   t_hx for tile i run in iteration i; the DVE tail (gd, y, bn_stats
   LayerNorm stats, normalize+store) for tile i runs in iteration i+1
   when all of its inputs are long ready.
 - psum: four 2-bank rings (gate, h, retr, softmax-smalls) with
   per-chunk granularity so each bank is drained by its one consumer
   right after its accumulation group stops.
 - x8t/xbt loads are chunked by 512-row groups, emitted lazily, and
   prefetched two chunks ahead; weights are ordered so tile 0's gate
   matmuls start while wo still streams.
 - the retrieval matmuls and the xn load for tile i+1 are emitted at the
   end of iteration i (their attn lhsT is ready a whole chunk early), so
   d always finds psum + xn ready at the next iteration's DVE head.
"""

import math
import sys

import numpy as np

try:
    import concourse.bass as bass
except ImportError:  # harness runs from a fresh dir; repo is baked in the image
    sys.path.insert(0, "/opt/trn_rl_repo")
    import concourse.bass as bass

import ml_dtypes

import concourse.mybir as mybir
import concourse.tile as tile
from concourse.bass_utils import run_bass_kernel_spmd
from concourse.masks import make_identity

# ---------------------------------------------------------------- constants
HID = 1024
SLOTS = 64
KD = 32
B = 16384
NCORES = 8
R = B // NCORES          # rows per core
P = 128                  # partitions
NT = R // P              # row-tiles per core
KO = HID // P            # k-chunks of the HID contraction
NB = 512                 # psum chunk width (one bank of fp32)
SCALE = 0.45 / math.sqrt(KD)
WS = 64.0                # fp8 weight scale (gate + h paths, attn scale)

F32 = mybir.dt.float32
BF16 = mybir.dt.bfloat16
F8E4 = mybir.dt.float8e4
npbf16 = ml_dtypes.bfloat16
npf8 = ml_dtypes.float8_e4m3fn

AF = mybir.ActivationFunctionType
OP = mybir.AluOpType
DR = mybir.MatmulPerfMode.DoubleRow

_nc_cache = {}


# ---------------------------------------------------------------- device IR
MAX_WAITS = 1


def _split_excess_waits(nc: bass.Bass, max_waits: int = MAX_WAITS):
    """This container's walrus build accepts only a couple of sem-wait slots
    per instruction ("Too many sync wait commands"), while Tile's
    sem-assigner happily attaches one wait per producer proc.  Hoist excess
    waits onto preceding NOPs on the same engine (engines execute their
    stream in order, so semantics are unchanged)."""
    n_split = 0
    for fn in nc.m.functions:
        for blk in fn.blocks:
            insts = list(blk.instructions)
            new = []
            changed = False
            for ins in insts:
                si = getattr(ins, "sync_info", None)
                waits = list(si.on_wait) if si is not None and si.on_wait else []
                if len(waits) > max_waits:
                    extra, keep = waits[:-max_waits], waits[-max_waits:]
                    for j in range(0, len(extra), max_waits):
                        nop = mybir.InstNoOp(
                            name=f"{ins.name}-w{j}",
                            engine=ins.engine,
                            bass_nofuse=True,
                            sync_info=mybir.SyncInfo(
                                on_wait=extra[j:j + max_waits], on_update=[]
                            ),
                        )
                        new.append(nop)
                    si.on_wait = keep
                    changed = True
                    n_split += 1
                new.append(ins)
            if changed:
                blk.instructions = new
    return n_split


def _build(has_affine: bool, repeat: int = 1) -> bass.Bass:
    nc = bass.Bass()

    NC4 = R // NB            # 512-row chunks per core

    x8t_d = nc.dram_tensor("x8t", [P, 4, 2, R], F8E4, kind="ExternalInput")
    wg8_d = nc.dram_tensor("wg8", [P, 2, 4, 2, NB], F8E4, kind="ExternalInput")
    kw8_d = nc.dram_tensor("kw8", [P, 4, 2, KD], F8E4, kind="ExternalInput")
    # h path: k < 256 contracts in fp8 DoubleRow (wo8), k >= 256 in bf16
    KH = 3 * KO // 4          # bf16 k-chunks of the h contraction
    xbt_d = nc.dram_tensor("xbt", [P * KH, R], BF16, kind="ExternalInput")
    wo_d = nc.dram_tensor("wo", [P * KH, HID], BF16, kind="ExternalInput")
    wo8_d = nc.dram_tensor("wo8", [P, 2, 1, 2, NB], F8E4, kind="ExternalInput")
    vm8_d = nc.dram_tensor("vm8", [KD, 2, 3 * HID], F8E4, kind="ExternalInput")
    kwpa_d = nc.dram_tensor("kwpa", [KD + 1, SLOTS], BF16, kind="ExternalInput")
    xn_d = nc.dram_tensor("xn", [R, HID], BF16, kind="ExternalInput")
    if has_affine:
        ln_d = nc.dram_tensor("lnw", [2, HID], F32, kind="ExternalInput")
    y_d = nc.dram_tensor("y", [R, HID], BF16, kind="ExternalOutput")

    with tile.TileContext(nc) as tc:
        with (
            tc.tile_pool(name="consts", bufs=1) as consts,
            tc.tile_pool(name="inx", bufs=3) as inx,
            tc.tile_pool(name="work", bufs=2) as work,
            tc.tile_pool(name="small", bufs=3) as small,
            tc.tile_pool(name="outp", bufs=3) as outp,
            tc.tile_pool(name="ps_gate", bufs=2, space="PSUM") as ps_gate,
            tc.tile_pool(name="ps_h", bufs=2, space="PSUM") as ps_h,
            tc.tile_pool(name="ps_rq", bufs=2, space="PSUM") as ps_rq,
            tc.tile_pool(name="ps_sm", bufs=2, space="PSUM") as ps_sm,
        ):
            # ---- x chunk loads first so tile 0's softmax starts asap -----
            x8t_s = consts.tile([P, 4, 2, R], F8E4)
            xbt_s = consts.tile([P, KH, R], BF16)
            xbt_ap = xbt_d[:, :].rearrange("(ko p) r -> p ko r", p=P)
            x_loaded = set()

            def ensure_x_chunk(c4):
                if c4 in x_loaded:
                    return
                x_loaded.add(c4)
                sl = slice(c4 * NB, (c4 + 1) * NB)
                nc.sync.dma_start(x8t_s[:, :, :, sl], x8t_d[:, :, :, sl])
                nc.sync.dma_start(xbt_s[:, :, sl], xbt_ap[:, :, sl])

            kw8_s = consts.tile([P, 4, 2, KD], F8E4)
            nc.sync.dma_start(kw8_s, kw8_d[:, :, :, :])

            # chunk 0: x8t first (qk path), xbt deferred past the weights
            # (the h matmuls that read it start much later)
            x_loaded.add(0)
            nc.sync.dma_start(x8t_s[:, :, :, 0:NB], x8t_d[:, :, :, 0:NB])

            kwpa_s = consts.tile([KD + 1, SLOTS], BF16)
            nc.sync.dma_start(kwpa_s, kwpa_d[:, :])
            vm8_s = consts.tile([KD, 2, 3 * HID], F8E4)
            nc.sync.dma_start(vm8_s, vm8_d[:, :, :])
            wg8_s = consts.tile([P, 2, 4, 2, NB], F8E4)
            for c in range(2):
                nc.sync.dma_start(wg8_s[:, c, :, :, :], wg8_d[:, c, :, :, :])
            wo8_s = consts.tile([P, 2, 1, 2, NB], F8E4)
            nc.sync.dma_start(wo8_s, wo8_d[:, :, :, :, :])
            nc.sync.dma_start(xbt_s[:, :, 0:NB], xbt_ap[:, :, 0:NB])
            wo_s = consts.tile([P, KH, HID], BF16)
            wo_ap = wo_d[:, :].rearrange("(ko p) n -> p ko n", p=P)
            for ko in range(KH):
                nc.sync.dma_start(wo_s[:, ko, :], wo_ap[:, ko, :])

            if has_affine:
                g_s = consts.tile([P, HID], F32)
                nc.gpsimd.dma_start(g_s, bass.AP(ln_d, 0, [[0, P], [1, HID]]))
                b_s = consts.tile([P, HID], F32)
                nc.gpsimd.dma_start(b_s, bass.AP(ln_d, HID, [[0, P], [1, HID]]))

            ident_s = consts.tile([P, P], BF16)
            make_identity(nc, ident_s)
            ident8_s = consts.tile([P, P], F8E4)
            make_identity(nc, ident8_s)

            ones_s = consts.tile([KD, 1], BF16)
            nc.vector.memset(ones_s, 1.0)

            epsl_s = consts.tile([P, 1], F32)
            nc.vector.memset(epsl_s, 1e-5)

            ln64_s = consts.tile([P, 1], F32)
            nc.vector.memset(ln64_s, math.log(WS))

            # ---- qk^T, batched in 512-row chunks, emitted lazily --------
            # qkta[kd, r] = sum_k key_w[kd, k] * x[r, k];  row KD = ||qk||/SCALE
            qkta = consts.tile([KD + 1, R], BF16, tag="qkta")
            sq_full = consts.tile([KD, R], BF16, tag="sq_full")
            qkt_done = set()

            def ensure_qkt_chunk(c4):
                if c4 in qkt_done:
                    return
                qkt_done.add(c4)
                ensure_x_chunk(c4)
                for pf in (c4 + 1, c4 + 2):   # prefetch x chunks ahead
                    if pf < NC4:
                        ensure_x_chunk(pf)
                sl = slice(c4 * NB, (c4 + 1) * NB)
                qkt_ps = ps_sm.tile([KD, NB], F32, tag="pss")
                for t in range(4):
                    nc.tensor.matmul(
                        qkt_ps, kw8_s[:, t, :, :], x8t_s[:, t, :, sl],
                        start=(t == 0), stop=(t == 3), perf_mode=DR,
                    )
                nc.scalar.activation(qkta[0:KD, sl], qkt_ps, AF.Copy,
                                     scale=1.0 / WS)
                nc.vector.tensor_mul(sq_full[:, sl], qkta[0:KD, sl],
                                     qkta[0:KD, sl])

            # ---- per row-tile softmax (pipelined ahead) -----------------
            # ---- softmax chain, batched per 512-row chunk (4 tiles) -----
            # A: 4 ss matmuls + one ACT sqrt + one DVE recip
            # B: 4 srt^T PE transposes + one DVE row copy
            # C: 4 sim matmuls + 4 ACT exp + DVE recip + 4 ACT quantizes
            # D: 8 fp8 PE transposes + ACT copies -> DoubleRow lhsT
            # One stage per iteration, spread over the 4 iterations of the
            # preceding chunk: every cross-engine round-trip has a whole
            # iteration of slack, so nothing convoys the engine queues.
            def ch_a(c):
                ensure_qkt_chunk(c)
                ss_ps = ps_sm.tile([P, 4], F32, tag="pss")
                for j in range(4):
                    rows = slice(c * NB + j * P, c * NB + (j + 1) * P)
                    nc.tensor.matmul(ss_ps[:, j:j + 1], sq_full[:, rows],
                                     ones_s, start=True, stop=True)
                # srt = ||qk|| / SCALE (bf16; its reciprocal is the row scale)
                srt4 = small.tile([P, 4], BF16, tag="sm_srt")
                nc.scalar.activation(
                    srt4, ss_ps, AF.Sqrt, scale=1.0 / (SCALE * SCALE)
                )
                sr4 = small.tile([P, 4], F32, tag="sm_sr")
                nc.vector.reciprocal(sr4, srt4)
                return {"c": c, "srt4": srt4, "sr4": sr4}

            def ch_b(st):
                c = st["c"]
                srtT_ps = ps_sm.tile([1, NB], BF16, tag="pss")
                for j in range(4):
                    nc.tensor.transpose(srtT_ps[:, j * P:(j + 1) * P],
                                        st["srt4"][:, j:j + 1], ident_s)
                nc.scalar.activation(
                    qkta[KD:KD + 1, c * NB:(c + 1) * NB], srtT_ps, AF.Copy
                )

            def ch_c_mm(st):
                sim_ps = ps_sm.tile([P, 4, SLOTS], F32, tag="pss")
                c = st["c"]
                for j in range(4):
                    rows = slice(c * NB + j * P, c * NB + (j + 1) * P)
                    nc.tensor.matmul(sim_ps[:, j, :], qkta[:, rows],
                                     kwpa_s, start=True, stop=True)
                st["sim_ps"] = sim_ps

            def ch_c_act(st):
                # attn_u = 64*exp(s_r*sim) via bias=ln(64); den accumulates
                # the x64 sum; rec = 1/(den/64) = 64/den; quantize is an
                # ACT Copy with the per-row scale ptr.  Emitted at the END
                # of the iteration so the tiny DVE hops (dens, rec) sit
                # behind the ready-to-run epilogue tail, not in front.
                sim_ps = st["sim_ps"]
                attn_u = small.tile([P, 4, SLOTS], BF16, tag="sm_attnu")
                den4 = small.tile([P, 4], F32, tag="sm_den")
                for j in range(4):
                    nc.scalar.activation(attn_u[:, j, :], sim_ps[:, j, :],
                                         AF.Exp, scale=st["sr4"][:, j:j + 1],
                                         bias=ln64_s,
                                         accum_out=den4[:, j:j + 1])
                dens = small.tile([P, 4], F32, tag="sm_dens")
                nc.vector.tensor_scalar_mul(dens, den4, 1.0 / WS)
                rec4 = small.tile([P, 4], F32, tag="sm_rec")
                nc.vector.reciprocal(rec4, dens)
                attn8 = small.tile([P, 4, SLOTS], F8E4, tag="sm_attn8")
                for j in range(4):
                    nc.scalar.activation(attn8[:, j, :], attn_u[:, j, :],
                                         AF.Copy, scale=rec4[:, j:j + 1])
                st["attn8"] = attn8

            def ch_d(st):
                """transpose fp8 attn into the DoubleRow lhsT layout
                [KD, 4, 2, P] with slot = 32*i + p.  The HW fp8 transpose
                writes with an element step of 2, so the psum staging tile
                carries an extra interleave dim that the ACT copy compacts."""
                attn8 = st["attn8"]
                attnt_ps = ps_sm.tile([KD, 4, 2, P, 2], F8E4, tag="pss")
                for j in range(4):
                    for i in range(2):
                        nc.tensor.transpose(
                            attnt_ps[:, j, i, :, 0],
                            attn8[:, j, KD * i:KD * (i + 1)], ident8_s,
                        )
                attnt_s = small.tile([KD, 4, 2, P], F8E4, tag="sm_attnt")
                nc.scalar.activation(attnt_s[:, 0:2, :, :],
                                     attnt_ps[:, 0:2, :, :, 0], AF.Copy)
                nc.scalar.activation(attnt_s[:, 2:4, :, :],
                                     attnt_ps[:, 2:4, :, :, 0], AF.Copy)
                return attnt_s

            def ch_full(c):
                st = ch_a(c)
                ch_b(st)
                ch_c_mm(st)
                ch_c_act(st)
                return st, ch_d(st)

            def tail1(tl):
                """Epilogue tail (part 1) for the PREVIOUS tile: every
                input is already computed, so these DVE ops never stall."""
                gd = work.tile([P, HID], BF16, tag="gd")
                nc.vector.tensor_mul(gd, tl["gate"], tl["d"])
                y = work.tile([P, HID], BF16, tag="y")
                nc.vector.tensor_add(y, gd, tl["t_hx"])

                # LayerNorm stats in one DVE pass: bn_stats halves + aggr
                bst = small.tile([P, 2, 6], F32, tag="sm_bst")
                nc.vector.bn_stats(bst[:, 0, :], y[:, 0:NB])
                nc.vector.bn_stats(bst[:, 1, :], y[:, NB:HID])
                mv = small.tile([P, 2], F32, tag="sm_mv")
                nc.vector.bn_aggr(mv, bst)
                tl["y"] = y
                tl["mv"] = mv

            def tail2(tl):
                """Normalize + store (emitted one iteration later, when
                the ACT stdv from the previous iteration has drained)."""
                mv = tl["mv"]
                stdv = small.tile([P, 1], F32, tag="sm_std")
                nc.scalar.activation(
                    stdv, mv[:, 1:2], AF.Sqrt, bias=epsl_s
                )
                rstd = small.tile([P, 1], F32, tag="sm_rstd")
                nc.vector.reciprocal(rstd, stdv)

                out_t = outp.tile([P, HID], BF16)
                nc.vector.tensor_scalar(
                    out_t, tl["y"], mv[:, 0:1], rstd, OP.subtract, OP.mult
                )
                if has_affine:
                    nc.vector.tensor_mul(out_t, out_t, g_s)
                    nc.vector.tensor_add(out_t, out_t, b_s)

                nc.sync.dma_start(y_d[tl["rows"], :], out_t)

            tiles = [t for _ in range(repeat) for t in range(NT)]
            n = len(tiles)
            ngroups = n // 4
            # Warmup: the full chain for chunk group 0 runs up front (its
            # qkt work overlaps the big const DMAs).
            _, attnt_grp = ch_full(tiles[0] * P // NB)
            st_next, attnt_next = None, None
            def emit_retr(attnt_j):
                """Retrieval matmuls (x64 in psum) for the NEXT tile,
                emitted at the end of the preceding iteration: the psum
                bank was freed by d at that iteration's DVE head, and d
                for the next tile finds its input ready immediately."""
                rch = []
                for c in range(2):
                    rp_c = ps_rq.tile([P, NB], F32, tag="psr")
                    nc.tensor.matmul(
                        rp_c, attnt_j,
                        vm8_s[:, :, c * NB:(c + 1) * NB],
                        start=True, stop=True, perf_mode=DR,
                    )
                    rch.append(rp_c)
                return rch

            def emit_xn(j):
                xn_t = inx.tile([P, HID], BF16)
                nc.sync.dma_start(xn_t, xn_d[j * P:(j + 1) * P, :])
                return xn_t

            pend_tail = None
            xn_t = emit_xn(tiles[0])
            rch = emit_retr(attnt_grp[:, 0, :, :])
            for idx, i in enumerate(tiles):
                rows = slice(i * P, (i + 1) * P)
                phase = idx % 4
                if phase == 0 and idx > 0:
                    attnt_grp = attnt_next
                attnt_cur = attnt_grp[:, phase, :, :]

                d = work.tile([P, HID], BF16, tag="d")
                for c in range(2):
                    csl = slice(c * NB, (c + 1) * NB)
                    nc.vector.scalar_tensor_tensor(
                        d[:, csl], rch[c], 1.0 / WS, xn_t[:, csl],
                        OP.mult, OP.subtract
                    )

                # one chain stage per iteration for the NEXT chunk group
                grp = idx // 4
                if phase == 0:
                    st_next = (ch_a(tiles[(grp + 1) * 4] * P // NB)
                               if grp + 1 < ngroups else None)
                elif st_next is not None and phase == 1:
                    ch_b(st_next)
                elif st_next is not None and phase == 2:
                    ch_c_mm(st_next)

                # ---- gate preact: fp8 DoubleRow x-part + attn-part ------
                gch = []
                for c in range(2):
                    gp_c = ps_gate.tile([P, NB], F32, tag="psg")
                    for t in range(4):
                        nc.tensor.matmul(
                            gp_c, x8t_s[:, t, :, rows],
                            wg8_s[:, c, t, :, :],
                            start=(t == 0), stop=False, perf_mode=DR,
                        )
                    nc.tensor.matmul(
                        gp_c, attnt_cur,
                        vm8_s[:, :, HID + c * NB:HID + (c + 1) * NB],
                        start=False, stop=True, perf_mode=DR,
                    )
                    gch.append(gp_c)

                gate = work.tile([P, HID], BF16, tag="gate")
                nc.scalar.activation(gate[:, 0:NB], gch[0], AF.Sigmoid,
                                     scale=1.0 / WS)
                nc.scalar.activation(gate[:, NB:HID], gch[1], AF.Sigmoid,
                                     scale=1.0 / WS)

                # ---- epilogue tail pt 1 for the previous tile -----------
                if pend_tail is not None:
                    tail1(pend_tail)

                # ---- h preact: fp8 DoubleRow for k<512 + bf16 for k>=512
                # (both x64) + fp8 attn-part
                hch = []
                xt = xbt_s[:, :, rows]
                for c in range(2):
                    csl = slice(c * NB, (c + 1) * NB)
                    hp_c = ps_h.tile([P, NB], F32, tag="psh")
                    nc.tensor.matmul(
                        hp_c, x8t_s[:, 0, :, rows],
                        wo8_s[:, c, 0, :, :],
                        start=True, stop=False, perf_mode=DR,
                    )
                    for k in range(KH):
                        nc.tensor.matmul(
                            hp_c, xt[:, k, :],
                            wo_s[:, k, csl],
                            start=False, stop=False,
                        )
                    nc.tensor.matmul(
                        hp_c, attnt_cur,
                        vm8_s[:, :, 2 * HID + c * NB:2 * HID + (c + 1) * NB],
                        start=False, stop=True, perf_mode=DR,
                    )
                    hch.append(hp_c)
                    if c == 0 and phase == 3 and st_next is not None:
                        attnt_next = ch_d(st_next)

                # ---- epilogue head (this tile) --------------------------
                h = work.tile([P, HID], BF16, tag="h")
                nc.scalar.activation(h[:, 0:NB], hch[0], AF.Gelu,
                                     scale=1.0 / WS)
                nc.scalar.activation(h[:, NB:HID], hch[1], AF.Gelu,
                                     scale=1.0 / WS)

                t_hx = work.tile([P, HID], BF16, tag="t_hx")
                if idx + 1 < n:
                    for c in range(2):
                        csl = slice(c * NB, (c + 1) * NB)
                        nc.gpsimd.tensor_add(t_hx[:, csl], h[:, csl],
                                             xn_t[:, csl])
                else:
                    # final tile: Pool's ~2.2us op would sit on the drain
                    # critical path; DVE does it in 0.6us
                    nc.vector.tensor_add(t_hx, h, xn_t)

                # ---- epilogue tail pt 2 for the previous tile -----------
                if pend_tail is not None:
                    tail2(pend_tail)
                pend_tail = {"rows": rows, "gate": gate, "d": d,
                             "t_hx": t_hx}

                # exp/quantize block for the next chunk at iteration end
                if st_next is not None and phase == 2:
                    ch_c_act(st_next)

                # next tile's xn load + retrieval matmuls (see emit_retr)
                if idx + 1 < n:
                    nphase = (idx + 1) % 4
                    agrp = attnt_grp if nphase != 0 else attnt_next
                    xn_t = emit_xn(tiles[idx + 1])
                    rch = emit_retr(agrp[:, nphase, :, :])

            tail1(pend_tail)
            tail2(pend_tail)

    _split_excess_waits(nc)
    return nc


def _get_nc(has_affine: bool) -> bass.Bass:
    key = has_affine
    if key not in _nc_cache:
        _nc_cache[key] = _build(has_affine)
    return _nc_cache[key]


# ---------------------------------------------------------------- host side
def _prep(x, key_w, out_w, out_b, gate_w, gate_b, ln_g, ln_b,
          pos_table, mem_keys, mem_vals, mem_age, mem_conf, slot_order):
    f32 = np.float32
    x = np.asarray(x, f32)
    key_w = np.asarray(key_w, f32)
    out_w = np.asarray(out_w, f32)
    out_b = np.asarray(out_b, f32)
    gate_w = np.asarray(gate_w, f32)
    gate_b = np.asarray(gate_b, f32)
    ln_g = np.asarray(ln_g, f32)
    ln_b = np.asarray(ln_b, f32)
    pos_table = np.asarray(pos_table, f32)
    mem_keys = np.asarray(mem_keys, f32)
    mem_vals = np.asarray(mem_vals, f32)
    mem_age = np.asarray(mem_age, f32)
    mem_conf = np.asarray(mem_conf, f32)
    slot_order = np.asarray(slot_order)

    # v3 folds the gate/out biases into nothing: they must be zero (they are,
    # per the problem spec).  Guard so a different harness fails loudly.
    assert np.all(gate_b == 0.0) and np.all(out_b == 0.0), \
        "v3 kernel assumes zero gate/out biases"

    has_affine = not (np.all(ln_g == 1.0) and np.all(ln_b == 0.0))

    def q8(a):
        return np.clip(a, -240.0, 240.0).astype(npf8)

    # gate x-part weights, fp8 DR layout [P, nchunk, 4, 2, NB]:
    # element (p, c, t, i, n) = WS * wg[256t + 128i + p, c*NB + n]
    wg = np.ascontiguousarray(gate_w[:, :HID].T)          # [HID, HID]
    wg8 = q8(
        (WS * wg).reshape(4, 2, P, 2, NB).transpose(2, 3, 0, 1, 4)
    )
    # h x-part weights, host-scaled x64 to match the fp8 attn-part.
    # k < 256 runs in fp8 DoubleRow (wg8-style layout), k >= 256 in bf16.
    wo_full = WS * np.ascontiguousarray(out_w[:, :HID].T)  # [HID, HID]
    wo8 = q8(
        wo_full[:256].reshape(1, 2, P, 2, NB).transpose(2, 3, 0, 1, 4)
    )
    wo_b = wo_full[256:].astype(npbf16)                    # [768, HID]
    # key weights fp8 DR layout [P, 4, 2, KD] (scaled x64, unscaled at the
    # ACT psum->sbuf copy together with the x8 contraction)
    kw8 = q8(
        (WS * key_w.T).reshape(4, 2, P, KD).transpose(2, 0, 1, 3)
    )

    mg = mem_vals @ gate_w[:, HID:].T                     # [S, HID]
    mo = mem_vals @ out_w[:, HID:].T
    # attn-side values, fp8 DoubleRow rhs layout [KD, 2, 3*HID]:
    # element (p, i, n) = vm[32*i + p, n]
    vm = np.concatenate([mem_vals, mg, mo], axis=1)       # [S, 3*HID]
    vm8 = q8(vm.reshape(2, KD, 3 * HID).transpose(1, 0, 2))

    pos_emb = pos_table[slot_order % SLOTS]
    kwp = mem_keys + f32(0.1) * pos_emb
    kwp = kwp / np.clip(
        np.linalg.norm(kwp, axis=-1, keepdims=True), 1e-12, None
    ).astype(f32)

    recency = np.exp(-mem_age / f32(200.0))
    freq = np.clip(mem_age, 1.0, None).astype(f32)
    freq_norm = np.log(freq + f32(1.0)) / (np.log(freq.max() + f32(2.0)) + f32(1e-8))
    biasv = (
        f32(0.2) * recency + f32(0.15) * freq_norm
        + f32(0.1) * mem_conf + f32(0.1) * f32(0.8)
    ).astype(f32)
    # |0.45*sim| <= 0.45/sqrt(KD) = 0.0796 exactly (normalized vectors), so
    # the clip(0,1) in the reference is provably inactive iff:
    simmax = 0.45 / math.sqrt(KD)
    assert biasv.min() > simmax and biasv.max() < 1.0 - simmax, \
        "salience clip would bind; v3 drops it"

    kwpa = np.concatenate(
        [np.ascontiguousarray(kwp.T), biasv[None, :]], axis=0
    ).astype(npbf16)                                      # [KD+1, S]

    xbt = np.ascontiguousarray(x.T[256:]).astype(npbf16)  # [768, B]
    # x fp8 DR layout [P, 4, 2, B]: element (p, t, i, r) = x[r, 256t+128i+p]
    x8t_full = q8(
        np.asarray(x.T).reshape(4, 2, P, B).transpose(2, 0, 1, 3)
    )                                                     # [P, 4, 2, B]
    xn = x.astype(npbf16)                                 # [B, HID]

    lnw = np.stack([ln_g, ln_b]).astype(f32) if has_affine else None

    in_maps = []
    for c in range(NCORES):
        rs, re = c * R, (c + 1) * R
        m = {
            "x8t": np.ascontiguousarray(x8t_full[:, :, :, rs:re]),
            "wg8": wg8,
            "kw8": kw8,
            "xbt": np.ascontiguousarray(xbt[:, rs:re]),
            "wo": wo_b,
            "wo8": wo8,
            "vm8": vm8,
            "kwpa": kwpa,
            "xn": np.ascontiguousarray(xn[rs:re]),
        }
        if has_affine:
            m["lnw"] = lnw
        in_maps.append(m)
    return in_maps, has_affine


def _run(trace=False, **inputs):
    in_maps, has_affine = _prep(**inputs)
    nc = _get_nc(has_affine)
    try:
        res = run_bass_kernel_spmd(
            nc, in_maps, core_ids=list(range(NCORES)), trace=trace
        )
    except Exception:
        # transient axon/NRT hiccups have been observed; one retry
        res = run_bass_kernel_spmd(
            nc, in_maps, core_ids=list(range(NCORES)), trace=trace
        )
    out = np.concatenate(
        [np.asarray(res.results[c]["y"]).astype(np.float32)
         for c in range(NCORES)], axis=0
    )
    return out, res


def kernel(**inputs) -> np.ndarray:
    out, _ = _run(trace=False, **inputs)
    return out


# revision 89
# speedup vs baseline: 1.1599x; 1.1599x over previous
"""Bass/Trainium2 kernel for nn_EpisodicMemory (8-core data-parallel), v4.

Batch (16384 rows) is sharded across 8 NeuronCores (2048 rows each, 16
row-tiles of 128).  All O(weights) transforms happen on the host; all O(B)
work happens on-device.

v4 changes vs v2 (TimelineSim per-core estimate 153.9us -> 109.2us):
 - every attn-side matmul (retrieval, gate attn-part, h attn-part) runs in
   fp8-e4m3 DoubleRow: the normalized attention is quantized to fp8 with a
   x64 scale (values ~1.0, comfortably in e4m3 normal range), mem_vals /
   mem_vals@w projections stay unit-scale fp8, and the x-part weights are
   host-scaled x64 so both psum contributions share the x64 factor, which
   the sigmoid/gelu ACT un-scales for free.
 - a quarter of the h-path contraction (k < 256) also runs fp8 DoubleRow
   (end-to-end rel err 1.57e-2 vs the 2e-2 gate; half-fp8 measured
   1.96e-2 - too thin a margin).
 - the softmax chain is batched per 512-row chunk (4 tiles) and split
   into four stages (ss/sqrt, srt^T/bias-row, sim/exp/quantize,
   fp8-transposes), one stage per iteration of the PRECEDING chunk, so
   every cross-engine round-trip has a full iteration of slack and never
   convoys the in-order engine queues.  The fp8 transpose writes with the
   HW-mandated element step of 2; an ACT copy compacts it.
 - epilogue splits across iterations: matmuls + sigmoid/gelu + Pool
   t_hx for tile i run in iteration i; the DVE tail (gd, y, bn_stats
   LayerNorm stats, normalize+store) for tile i runs in iteration i+1
   when all of its inputs are long ready.
 - psum: four 2-bank rings (gate, h, retr, softmax-smalls) with
   per-chunk granularity so each bank is drained by its one consumer
   right after its accumulation group stops.
 - x8t/xbt loads are chunked by 512-row groups, emitted lazily, and
   prefetched two chunks ahead; weights are ordered so tile 0's gate
   matmuls start while wo still streams.
 - the retrieval matmuls and the xn load for tile i+1 are emitted at the
   end of iteration i (their attn lhsT is ready a whole chunk early), so
   d always finds psum + xn ready at the next iteration's DVE head.
 - the final tile's t_hx runs on DVE (0.6us) instead of Pool (2.2us),
   which sits on the end-of-kernel drain critical path.
"""

import math
import sys

import numpy as np

try:
    import concourse.bass as bass
except ImportError:  # harness runs from a fresh dir; repo is baked in the image
    sys.path.insert(0, "/opt/trn_rl_repo")
    import concourse.bass as bass

import ml_dtypes

import concourse.mybir as mybir
import concourse.tile as tile
from concourse.bass_utils import run_bass_kernel_spmd
from concourse.masks import make_identity

# ---------------------------------------------------------------- constants
HID = 1024
SLOTS = 64
KD = 32
B = 16384
NCORES = 8
R = B // NCORES          # rows per core
P = 128                  # partitions
NT = R // P              # row-tiles per core
KO = HID // P            # k-chunks of the HID contraction
NB = 512                 # psum chunk width (one bank of fp32)
SCALE = 0.45 / math.sqrt(KD)
WS = 64.0                # fp8 weight scale (gate + h paths, attn scale)

F32 = mybir.dt.float32
BF16 = mybir.dt.bfloat16
F8E4 = mybir.dt.float8e4
npbf16 = ml_dtypes.bfloat16
npf8 = ml_dtypes.float8_e4m3fn

AF = mybir.ActivationFunctionType
OP = mybir.AluOpType
DR = mybir.MatmulPerfMode.DoubleRow

_nc_cache = {}


# ---------------------------------------------------------------- device IR
MAX_WAITS = 1


def _split_excess_waits(nc: bass.Bass, max_waits: int = MAX_WAITS):
    """This container's walrus build accepts only a couple of sem-wait slots
    per instruction ("Too many sync wait commands"), while Tile's
    sem-assigner happily attaches one wait per producer proc.  Hoist excess
    waits onto preceding NOPs on the same engine (engines execute their
    stream in order, so semantics are unchanged)."""
    n_split = 0
    for fn in nc.m.functions:
        for blk in fn.blocks:
            insts = list(blk.instructions)
            new = []
            changed = False
            for ins in insts:
                si = getattr(ins, "sync_info", None)
                waits = list(si.on_wait) if si is not None and si.on_wait else []
                if len(waits) > max_waits:
                    extra, keep = waits[:-max_waits], waits[-max_waits:]
                    for j in range(0, len(extra), max_waits):
                        nop = mybir.InstNoOp(
                            name=f"{ins.name}-w{j}",
                            engine=ins.engine,
                            bass_nofuse=True,
                            sync_info=mybir.SyncInfo(
                                on_wait=extra[j:j + max_waits], on_update=[]
                            ),
                        )
                        new.append(nop)
                    si.on_wait = keep
                    changed = True
                    n_split += 1
                new.append(ins)
            if changed:
                blk.instructions = new
    return n_split


def _build(has_affine: bool, repeat: int = 1) -> bass.Bass:
    nc = bass.Bass()

    NC4 = R // NB            # 512-row chunks per core

    x8t_d = nc.dram_tensor("x8t", [P, 4, 2, R], F8E4, kind="ExternalInput")
    wg8_d = nc.dram_tensor("wg8", [P, 2, 4, 2, NB], F8E4, kind="ExternalInput")
    kw8_d = nc.dram_tensor("kw8", [P, 4, 2, KD], F8E4, kind="ExternalInput")
    # h path: k < 256 contracts in fp8 DoubleRow (wo8), k >= 256 in bf16
    KH = 3 * KO // 4          # bf16 k-chunks of the h contraction
    xbt_d = nc.dram_tensor("xbt", [P * KH, R], BF16, kind="ExternalInput")
    wo_d = nc.dram_tensor("wo", [P * KH, HID], BF16, kind="ExternalInput")
    wo8_d = nc.dram_tensor("wo8", [P, 2, 1, 2, NB], F8E4, kind="ExternalInput")
    vm8_d = nc.dram_tensor("vm8", [KD, 2, 3 * HID], F8E4, kind="ExternalInput")
    kwpa_d = nc.dram_tensor("kwpa", [KD + 1, SLOTS], BF16, kind="ExternalInput")
    xn_d = nc.dram_tensor("xn", [R, HID], BF16, kind="ExternalInput")
    if has_affine:
        ln_d = nc.dram_tensor("lnw", [2, HID], F32, kind="ExternalInput")
    y_d = nc.dram_tensor("y", [R, HID], BF16, kind="ExternalOutput")

    with tile.TileContext(nc) as tc:
        with (
            tc.tile_pool(name="consts", bufs=1) as consts,
            tc.tile_pool(name="inx", bufs=3) as inx,
            tc.tile_pool(name="work", bufs=2) as work,
            tc.tile_pool(name="small", bufs=3) as small,
            tc.tile_pool(name="outp", bufs=3) as outp,
            tc.tile_pool(name="ps_gate", bufs=2, space="PSUM") as ps_gate,
            tc.tile_pool(name="ps_h", bufs=2, space="PSUM") as ps_h,
            tc.tile_pool(name="ps_rq", bufs=2, space="PSUM") as ps_rq,
            tc.tile_pool(name="ps_sm", bufs=2, space="PSUM") as ps_sm,
        ):
            # ---- x chunk loads first so tile 0's softmax starts asap -----
            x8t_s = consts.tile([P, 4, 2, R], F8E4)
            xbt_s = consts.tile([P, KH, R], BF16)
            xbt_ap = xbt_d[:, :].rearrange("(ko p) r -> p ko r", p=P)
            x_loaded = set()

            def ensure_x_chunk(c4):
                if c4 in x_loaded:
                    return
                x_loaded.add(c4)
                sl = slice(c4 * NB, (c4 + 1) * NB)
                nc.sync.dma_start(x8t_s[:, :, :, sl], x8t_d[:, :, :, sl])
                nc.sync.dma_start(xbt_s[:, :, sl], xbt_ap[:, :, sl])

            kw8_s = consts.tile([P, 4, 2, KD], F8E4)
            nc.sync.dma_start(kw8_s, kw8_d[:, :, :, :])

            # chunk 0: x8t first (qk path), xbt deferred past the weights
            # (the h matmuls that read it start much later)
            x_loaded.add(0)
            nc.sync.dma_start(x8t_s[:, :, :, 0:NB], x8t_d[:, :, :, 0:NB])

            kwpa_s = consts.tile([KD + 1, SLOTS], BF16)
            nc.sync.dma_start(kwpa_s, kwpa_d[:, :])
            vm8_s = consts.tile([KD, 2, 3 * HID], F8E4)
            nc.sync.dma_start(vm8_s, vm8_d[:, :, :])
            wg8_s = consts.tile([P, 2, 4, 2, NB], F8E4)
            for c in range(2):
                nc.sync.dma_start(wg8_s[:, c, :, :, :], wg8_d[:, c, :, :, :])
            wo8_s = consts.tile([P, 2, 1, 2, NB], F8E4)
            nc.sync.dma_start(wo8_s, wo8_d[:, :, :, :, :])
            nc.sync.dma_start(xbt_s[:, :, 0:NB], xbt_ap[:, :, 0:NB])
            wo_s = consts.tile([P, KH, HID], BF16)
            wo_ap = wo_d[:, :].rearrange("(ko p) n -> p ko n", p=P)
            for ko in range(KH):
                nc.sync.dma_start(wo_s[:, ko, :], wo_ap[:, ko, :])

            if has_affine:
                g_s = consts.tile([P, HID], F32)
                nc.gpsimd.dma_start(g_s, bass.AP(ln_d, 0, [[0, P], [1, HID]]))
                b_s = consts.tile([P, HID], F32)
                nc.gpsimd.dma_start(b_s, bass.AP(ln_d, HID, [[0, P], [1, HID]]))

            ident_s = consts.tile([P, P], BF16)
            make_identity(nc, ident_s)
            ident8_s = consts.tile([P, P], F8E4)
            make_identity(nc, ident8_s)

            ones_s = consts.tile([KD, 1], BF16)
            nc.vector.memset(ones_s, 1.0)

            epsl_s = consts.tile([P, 1], F32)
            nc.vector.memset(epsl_s, 1e-5)

            ln64_s = consts.tile([P, 1], F32)
            nc.vector.memset(ln64_s, math.log(WS))

            # ---- qk^T, batched in 512-row chunks, emitted lazily --------
            # qkta[kd, r] = sum_k key_w[kd, k] * x[r, k];  row KD = ||qk||/SCALE
            qkta = consts.tile([KD + 1, R], BF16, tag="qkta")
            sq_full = consts.tile([KD, R], BF16, tag="sq_full")
            qkt_done = set()

            def ensure_qkt_chunk(c4):
                if c4 in qkt_done:
                    return
                qkt_done.add(c4)
                ensure_x_chunk(c4)
                for pf in (c4 + 1, c4 + 2):   # prefetch x chunks ahead
                    if pf < NC4:
                        ensure_x_chunk(pf)
                sl = slice(c4 * NB, (c4 + 1) * NB)
                qkt_ps = ps_sm.tile([KD, NB], F32, tag="pss")
                for t in range(4):
                    nc.tensor.matmul(
                        qkt_ps, kw8_s[:, t, :, :], x8t_s[:, t, :, sl],
                        start=(t == 0), stop=(t == 3), perf_mode=DR,
                    )
                nc.scalar.activation(qkta[0:KD, sl], qkt_ps, AF.Copy,
                                     scale=1.0 / WS)
                nc.vector.tensor_mul(sq_full[:, sl], qkta[0:KD, sl],
                                     qkta[0:KD, sl])

            # ---- per row-tile softmax (pipelined ahead) -----------------
            # ---- softmax chain, batched per 512-row chunk (4 tiles) -----
            # A: 4 ss matmuls + one ACT sqrt + one DVE recip
            # B: 4 srt^T PE transposes + one DVE row copy
            # C: 4 sim matmuls + 4 ACT exp + DVE recip + 4 ACT quantizes
            # D: 8 fp8 PE transposes + ACT copies -> DoubleRow lhsT
            # One stage per iteration, spread over the 4 iterations of the
            # preceding chunk: every cross-engine round-trip has a whole
            # iteration of slack, so nothing convoys the engine queues.
            def ch_a(c):
                ensure_qkt_chunk(c)
                ss_ps = ps_sm.tile([P, 4], F32, tag="pss")
                for j in range(4):
                    rows = slice(c * NB + j * P, c * NB + (j + 1) * P)
                    nc.tensor.matmul(ss_ps[:, j:j + 1], sq_full[:, rows],
                                     ones_s, start=True, stop=True)
                # srt = ||qk|| / SCALE (bf16; its reciprocal is the row scale)
                srt4 = small.tile([P, 4], BF16, tag="sm_srt")
                nc.scalar.activation(
                    srt4, ss_ps, AF.Sqrt, scale=1.0 / (SCALE * SCALE)
                )
                sr4 = small.tile([P, 4], F32, tag="sm_sr")
                nc.vector.reciprocal(sr4, srt4)
                return {"c": c, "srt4": srt4, "sr4": sr4}

            def ch_b(st):
                c = st["c"]
                srtT_ps = ps_sm.tile([1, NB], BF16, tag="pss")
                for j in range(4):
                    nc.tensor.transpose(srtT_ps[:, j * P:(j + 1) * P],
                                        st["srt4"][:, j:j + 1], ident_s)
                nc.scalar.activation(
                    qkta[KD:KD + 1, c * NB:(c + 1) * NB], srtT_ps, AF.Copy
                )

            def ch_c_mm(st):
                sim_ps = ps_sm.tile([P, 4, SLOTS], F32, tag="pss")
                c = st["c"]
                for j in range(4):
                    rows = slice(c * NB + j * P, c * NB + (j + 1) * P)
                    nc.tensor.matmul(sim_ps[:, j, :], qkta[:, rows],
                                     kwpa_s, start=True, stop=True)
                st["sim_ps"] = sim_ps

            def ch_c_act(st):
                # attn_u = 64*exp(s_r*sim) via bias=ln(64); den accumulates
                # the x64 sum; rec = 1/(den/64) = 64/den; quantize is an
                # ACT Copy with the per-row scale ptr.  Emitted at the END
                # of the iteration so the tiny DVE hops (dens, rec) sit
                # behind the ready-to-run epilogue tail, not in front.
                sim_ps = st["sim_ps"]
                attn_u = small.tile([P, 4, SLOTS], BF16, tag="sm_attnu")
                den4 = small.tile([P, 4], F32, tag="sm_den")
                for j in range(4):
                    nc.scalar.activation(attn_u[:, j, :], sim_ps[:, j, :],
                                         AF.Exp, scale=st["sr4"][:, j:j + 1],
                                         bias=ln64_s,
                                         accum_out=den4[:, j:j + 1])
                dens = small.tile([P, 4], F32, tag="sm_dens")
                nc.vector.tensor_scalar_mul(dens, den4, 1.0 / WS)
                rec4 = small.tile([P, 4], F32, tag="sm_rec")
                nc.vector.reciprocal(rec4, dens)
                attn8 = small.tile([P, 4, SLOTS], F8E4, tag="sm_attn8")
                for j in range(4):
                    nc.scalar.activation(attn8[:, j, :], attn_u[:, j, :],
                                         AF.Copy, scale=rec4[:, j:j + 1])
                st["attn8"] = attn8

            def ch_d(st):
                """transpose fp8 attn into the DoubleRow lhsT layout
                [KD, 4, 2, P] with slot = 32*i + p.  The HW fp8 transpose
                writes with an element step of 2, so the psum staging tile
                carries an extra interleave dim that the ACT copy compacts."""
                attn8 = st["attn8"]
                attnt_ps = ps_sm.tile([KD, 4, 2, P, 2], F8E4, tag="pss")
                for j in range(4):
                    for i in range(2):
                        nc.tensor.transpose(
                            attnt_ps[:, j, i, :, 0],
                            attn8[:, j, KD * i:KD * (i + 1)], ident8_s,
                        )
                attnt_s = small.tile([KD, 4, 2, P], F8E4, tag="sm_attnt")
                nc.scalar.activation(attnt_s[:, 0:2, :, :],
                                     attnt_ps[:, 0:2, :, :, 0], AF.Copy)
                nc.scalar.activation(attnt_s[:, 2:4, :, :],
                                     attnt_ps[:, 2:4, :, :, 0], AF.Copy)
                return attnt_s

            def ch_full(c):
                st = ch_a(c)
                ch_b(st)
                ch_c_mm(st)
                ch_c_act(st)
                return st, ch_d(st)

            def tail1(tl):
                """Epilogue tail (part 1) for the PREVIOUS tile: every
                input is already computed, so these DVE ops never stall."""
                gd = work.tile([P, HID], BF16, tag="gd")
                nc.vector.tensor_mul(gd, tl["gate"], tl["d"])
                y = work.tile([P, HID], BF16, tag="y")
                nc.vector.tensor_add(y, gd, tl["t_hx"])

                # LayerNorm stats in one DVE pass: bn_stats halves + aggr
                bst = small.tile([P, 2, 6], F32, tag="sm_bst")
                nc.vector.bn_stats(bst[:, 0, :], y[:, 0:NB])
                nc.vector.bn_stats(bst[:, 1, :], y[:, NB:HID])
                mv = small.tile([P, 2], F32, tag="sm_mv")
                nc.vector.bn_aggr(mv, bst)
                tl["y"] = y
                tl["mv"] = mv

            def tail2(tl):
                """Normalize + store (emitted one iteration later, when
                the ACT stdv from the previous iteration has drained)."""
                mv = tl["mv"]
                stdv = small.tile([P, 1], F32, tag="sm_std")
                nc.scalar.activation(
                    stdv, mv[:, 1:2], AF.Sqrt, bias=epsl_s
                )
                rstd = small.tile([P, 1], F32, tag="sm_rstd")
                nc.vector.reciprocal(rstd, stdv)

                out_t = outp.tile([P, HID], BF16)
                nc.vector.tensor_scalar(
                    out_t, tl["y"], mv[:, 0:1], rstd, OP.subtract, OP.mult
                )
                if has_affine:
                    nc.vector.tensor_mul(out_t, out_t, g_s)
                    nc.vector.tensor_add(out_t, out_t, b_s)

                nc.sync.dma_start(y_d[tl["rows"], :], out_t)

            tiles = [t for _ in range(repeat) for t in range(NT)]
            n = len(tiles)
            ngroups = n // 4
            # Warmup: the full chain for chunk group 0 runs up front (its
            # qkt work overlaps the big const DMAs).
            _, attnt_grp = ch_full(tiles[0] * P // NB)
            st_next, attnt_next = None, None
            def emit_retr(attnt_j):
                """Retrieval matmuls (x64 in psum) for the NEXT tile,
                emitted at the end of the preceding iteration: the psum
                bank was freed by d at that iteration's DVE head, and d
                for the next tile finds its input ready immediately."""
                rch = []
                for c in range(2):
                    rp_c = ps_rq.tile([P, NB], F32, tag="psr")
                    nc.tensor.matmul(
                        rp_c, attnt_j,
                        vm8_s[:, :, c * NB:(c + 1) * NB],
                        start=True, stop=True, perf_mode=DR,
                    )
                    rch.append(rp_c)
                return rch

            def emit_xn(j):
                xn_t = inx.tile([P, HID], BF16)
                nc.sync.dma_start(xn_t, xn_d[j * P:(j + 1) * P, :])
                return xn_t

            pend_tail = None
            xn_t = emit_xn(tiles[0])
            rch = emit_retr(attnt_grp[:, 0, :, :])
            for idx, i in enumerate(tiles):
                rows = slice(i * P, (i + 1) * P)
                phase = idx % 4
                if phase == 0 and idx > 0:
                    attnt_grp = attnt_next
                attnt_cur = attnt_grp[:, phase, :, :]

                d = work.tile([P, HID], BF16, tag="d")
                for c in range(2):
                    csl = slice(c * NB, (c + 1) * NB)
                    nc.vector.scalar_tensor_tensor(
                        d[:, csl], rch[c], 1.0 / WS, xn_t[:, csl],
                        OP.mult, OP.subtract
                    )

                # one chain stage per iteration for the NEXT chunk group
                grp = idx // 4
                if phase == 0:
                    st_next = (ch_a(tiles[(grp + 1) * 4] * P // NB)
                               if grp + 1 < ngroups else None)
                elif st_next is not None and phase == 1:
                    ch_b(st_next)
                elif st_next is not None and phase == 2:
                    ch_c_mm(st_next)

                # ---- gate preact: fp8 DoubleRow x-part + attn-part ------
                gch = []
                for c in range(2):
                    gp_c = ps_gate.tile([P, NB], F32, tag="psg")
                    for t in range(4):
                        nc.tensor.matmul(
                            gp_c, x8t_s[:, t, :, rows],
                            wg8_s[:, c, t, :, :],
                            start=(t == 0), stop=False, perf_mode=DR,
                        )
                    nc.tensor.matmul(
                        gp_c, attnt_cur,
                        vm8_s[:, :, HID + c * NB:HID + (c + 1) * NB],
                        start=False, stop=True, perf_mode=DR,
                    )
                    gch.append(gp_c)

                gate = work.tile([P, HID], BF16, tag="gate")
                nc.scalar.activation(gate[:, 0:NB], gch[0], AF.Sigmoid,
                                     scale=1.0 / WS)
                nc.scalar.activation(gate[:, NB:HID], gch[1], AF.Sigmoid,
                                     scale=1.0 / WS)

                # ---- epilogue tail pt 1 for the previous tile -----------
                if pend_tail is not None:
                    tail1(pend_tail)

                # ---- h preact: fp8 DoubleRow for k<512 + bf16 for k>=512
                # (both x64) + fp8 attn-part
                hch = []
                xt = xbt_s[:, :, rows]
                for c in range(2):
                    csl = slice(c * NB, (c + 1) * NB)
                    hp_c = ps_h.tile([P, NB], F32, tag="psh")
                    nc.tensor.matmul(
                        hp_c, x8t_s[:, 0, :, rows],
                        wo8_s[:, c, 0, :, :],
                        start=True, stop=False, perf_mode=DR,
                    )
                    for k in range(KH):
                        nc.tensor.matmul(
                            hp_c, xt[:, k, :],
                            wo_s[:, k, csl],
                            start=False, stop=False,
                        )
                    nc.tensor.matmul(
                        hp_c, attnt_cur,
                        vm8_s[:, :, 2 * HID + c * NB:2 * HID + (c + 1) * NB],
                        start=False, stop=True, perf_mode=DR,
                    )
                    hch.append(hp_c)
                    if c == 0 and phase == 3 and st_next is not None:
                        attnt_next = ch_d(st_next)

                # ---- epilogue head (this tile) --------------------------
                h = work.tile([P, HID], BF16, tag="h")
                nc.scalar.activation(h[:, 0:NB], hch[0], AF.Gelu,
                                     scale=1.0 / WS)
                nc.scalar.activation(h[:, NB:HID], hch[1], AF.Gelu,
                                     scale=1.0 / WS)

                t_hx = work.tile([P, HID], BF16, tag="t_hx")
                if idx + 1 < n:
                    for c in range(2):
                        csl = slice(c * NB, (c + 1) * NB)
                        nc.gpsimd.tensor_add(t_hx[:, csl], h[:, csl],
                                             xn_t[:, csl])
                else:
                    # final tile: Pool's ~2.2us op would sit on the drain
                    # critical path; DVE does it in 0.6us
                    nc.vector.tensor_add(t_hx, h, xn_t)

                # ---- epilogue tail pt 2 for the previous tile -----------
                if pend_tail is not None:
                    tail2(pend_tail)
                pend_tail = {"rows": rows, "gate": gate, "d": d,
                             "t_hx": t_hx}

                # exp/quantize block for the next chunk at iteration end
                if st_next is not None and phase == 2:
                    ch_c_act(st_next)

                # next tile's xn load + retrieval matmuls (see emit_retr)
                if idx + 1 < n:
                    nphase = (idx + 1) % 4
                    agrp = attnt_grp if nphase != 0 else attnt_next
                    xn_t = emit_xn(tiles[idx + 1])
                    rch = emit_retr(agrp[:, nphase, :, :])

            tail1(pend_tail)
            tail2(pend_tail)

    _split_excess_waits(nc)
    return nc


def _get_nc(has_affine: bool) -> bass.Bass:
    key = has_affine
    if key not in _nc_cache:
        _nc_cache[key] = _build(has_affine)
    return _nc_cache[key]


# ---------------------------------------------------------------- host side
def _prep(x, key_w, out_w, out_b, gate_w, gate_b, ln_g, ln_b,
          pos_table, mem_keys, mem_vals, mem_age, mem_conf, slot_order):
    f32 = np.float32
    x = np.asarray(x, f32)
    key_w = np.asarray(key_w, f32)
    out_w = np.asarray(out_w, f32)
    out_b = np.asarray(out_b, f32)
    gate_w = np.asarray(gate_w, f32)
    gate_b = np.asarray(gate_b, f32)
    ln_g = np.asarray(ln_g, f32)
    ln_b = np.asarray(ln_b, f32)
    pos_table = np.asarray(pos_table, f32)
    mem_keys = np.asarray(mem_keys, f32)
    mem_vals = np.asarray(mem_vals, f32)
    mem_age = np.asarray(mem_age, f32)
    mem_conf = np.asarray(mem_conf, f32)
    slot_order = np.asarray(slot_order)

    # v3 folds the gate/out biases into nothing: they must be zero (they are,
    # per the problem spec).  Guard so a different harness fails loudly.
    assert np.all(gate_b == 0.0) and np.all(out_b == 0.0), \
        "v3 kernel assumes zero gate/out biases"

    has_affine = not (np.all(ln_g == 1.0) and np.all(ln_b == 0.0))

    def q8(a):
        return np.clip(a, -240.0, 240.0).astype(npf8)

    # gate x-part weights, fp8 DR layout [P, nchunk, 4, 2, NB]:
    # element (p, c, t, i, n) = WS * wg[256t + 128i + p, c*NB + n]
    wg = np.ascontiguousarray(gate_w[:, :HID].T)          # [HID, HID]
    wg8 = q8(
        (WS * wg).reshape(4, 2, P, 2, NB).transpose(2, 3, 0, 1, 4)
    )
    # h x-part weights, host-scaled x64 to match the fp8 attn-part.
    # k < 256 runs in fp8 DoubleRow (wg8-style layout), k >= 256 in bf16.
    wo_full = WS * np.ascontiguousarray(out_w[:, :HID].T)  # [HID, HID]
    wo8 = q8(
        wo_full[:256].reshape(1, 2, P, 2, NB).transpose(2, 3, 0, 1, 4)
    )
    wo_b = wo_full[256:].astype(npbf16)                    # [768, HID]
    # key weights fp8 DR layout [P, 4, 2, KD] (scaled x64, unscaled at the
    # ACT psum->sbuf copy together with the x8 contraction)
    kw8 = q8(
        (WS * key_w.T).reshape(4, 2, P, KD).transpose(2, 0, 1, 3)
    )

    mg = mem_vals @ gate_w[:, HID:].T                     # [S, HID]
    mo = mem_vals @ out_w[:, HID:].T
    # attn-side values, fp8 DoubleRow rhs layout [KD, 2, 3*HID]:
    # element (p, i, n) = vm[32*i + p, n]
    vm = np.concatenate([mem_vals, mg, mo], axis=1)       # [S, 3*HID]
    vm8 = q8(vm.reshape(2, KD, 3 * HID).transpose(1, 0, 2))

    pos_emb = pos_table[slot_order % SLOTS]
    kwp = mem_keys + f32(0.1) * pos_emb
    kwp = kwp / np.clip(
        np.linalg.norm(kwp, axis=-1, keepdims=True), 1e-12, None
    ).astype(f32)

    recency = np.exp(-mem_age / f32(200.0))
    freq = np.clip(mem_age, 1.0, None).astype(f32)
    freq_norm = np.log(freq + f32(1.0)) / (np.log(freq.max() + f32(2.0)) + f32(1e-8))
    biasv = (
        f32(0.2) * recency + f32(0.15) * freq_norm
        + f32(0.1) * mem_conf + f32(0.1) * f32(0.8)
    ).astype(f32)
    # |0.45*sim| <= 0.45/sqrt(KD) = 0.0796 exactly (normalized vectors), so
    # the clip(0,1) in the reference is provably inactive iff:
    simmax = 0.45 / math.sqrt(KD)
    assert biasv.min() > simmax and biasv.max() < 1.0 - simmax, \
        "salience clip would bind; v3 drops it"

    kwpa = np.concatenate(
        [np.ascontiguousarray(kwp.T), biasv[None, :]], axis=0
    ).astype(npbf16)                                      # [KD+1, S]

    xbt = np.ascontiguousarray(x.T[256:]).astype(npbf16)  # [768, B]
    # x fp8 DR layout [P, 4, 2, B]: element (p, t, i, r) = x[r, 256t+128i+p]
    x8t_full = q8(
        np.asarray(x.T).reshape(4, 2, P, B).transpose(2, 0, 1, 3)
    )                                                     # [P, 4, 2, B]
    xn = x.astype(npbf16)                                 # [B, HID]

    lnw = np.stack([ln_g, ln_b]).astype(f32) if has_affine else None

    in_maps = []
    for c in range(NCORES):
        rs, re = c * R, (c + 1) * R
        m = {
            "x8t": np.ascontiguousarray(x8t_full[:, :, :, rs:re]),
            "wg8": wg8,
            "kw8": kw8,
            "xbt": np.ascontiguousarray(xbt[:, rs:re]),
            "wo": wo_b,
            "wo8": wo8,
            "vm8": vm8,
            "kwpa": kwpa,
            "xn": np.ascontiguousarray(xn[rs:re]),
        }
        if has_affine:
            m["lnw"] = lnw
        in_maps.append(m)
    return in_maps, has_affine


def _run(trace=False, **inputs):
    in_maps, has_affine = _prep(**inputs)
    nc = _get_nc(has_affine)
    try:
        res = run_bass_kernel_spmd(
            nc, in_maps, core_ids=list(range(NCORES)), trace=trace
        )
    except Exception:
        # transient axon/NRT hiccups have been observed; one retry
        res = run_bass_kernel_spmd(
            nc, in_maps, core_ids=list(range(NCORES)), trace=trace
        )
    out = np.concatenate(
        [np.asarray(res.results[c]["y"]).astype(np.float32)
         for c in range(NCORES)], axis=0
    )
    return out, res


def kernel(**inputs) -> np.ndarray:
    out, _ = _run(trace=False, **inputs)
    return out
